# revision 6
# baseline (speedup 1.0000x reference)
"""Trainium2 Bass kernel for nn_MetaComprehensiveRegularization.

reference math (fp32):
  loss_common  = -sum(zc * zc)
  loss_special = -sum_v sum_i dot(zc_i, zs_vi) / (||zc_i|| * ||zs_vi||)
  output = stack([loss_common, loss_special])  # shape [2]

Data-parallel shard along N across 8 cores; single HWDGE f32 input
stream (measured 362 GB/s, 5.2->63.2us), both vector engines saturated:
  DVE: 64 dots via scalar_tensor_tensor+accum (766ns/block)
       16 cn2 via bn_stats (681ns/block, no read-accumulator)
  ACT: 64 sn2 via Square+accum->PSUM (~1.0us/block), staged to SBUF
       with per-chunk copies.
The stream head interleaves zc0/t1 sub-DMAs so both engines start ~6us;
outputs are split early/late on the two HWDGE rings (sync + scalar) so
the final DMAs gate only on their producing engine. Host combines the
raw per-row stats in float64.
"""

from contextlib import ExitStack

import numpy as np

N_CORES = 8
N, D, V = 16384, 512, 4
N_LOC = N // N_CORES      # 2048 rows per core
P = 128                   # SBUF partitions
A = 4                     # rows per partition per chunk
BLOCKS = N_LOC // P       # 16 row-blocks per core
CHUNKS = BLOCKS // A      # 4 chunks; chunk = 1 zc tile + 4 zs tiles
NT = V * CHUNKS           # 16 zs tiles

# out_bn layout: 16 cn2 blocks x 6 bn values, block (c,a) at 6*(4c+a)
BN_COLS = 96
# out_f32 layout: per chunk 32 cols: [dots 4v+a : 16 | sn2 4v+a : 16]
F32_COLS = 32 * CHUNKS    # 128

_PROGRAM = None


def _build_program():
    import concourse.bacc as bacc
    from concourse import mybir

    f32 = mybir.dt.float32
    mult = mybir.AluOpType.mult
    nc = bacc.Bacc(
        "TRN2", target_bir_lowering=False, debug=False, num_devices=N_CORES
    )
    zc_t = nc.dram_tensor("zc", [N_LOC, D], f32, kind="ExternalInput")
    zs_t = nc.dram_tensor("zs", [V, N_LOC, D], f32, kind="ExternalInput")
    out_bn_t = nc.dram_tensor("out_bn", [P, BN_COLS], f32, kind="ExternalOutput")
    out_f32_t = nc.dram_tensor("out_f32", [P, F32_COLS], f32, kind="ExternalOutput")

    # row n = c*(A*P) + p*A + a -> view [c, p, a, d]: whole-tile DMAs walk the
    # HBM source fully sequentially.
    zc_v = zc_t.ap().rearrange("(c p a) d -> c p a d", a=A, p=P)
    zs_v = zs_t.ap().rearrange("v (c p a) d -> v c p a d", a=A, p=P)

    with ExitStack() as ctx:
        zc_sl = [
            ctx.enter_context(nc.sbuf_tensor(f"zc{c}", [P, A, D], f32))
            for c in range(CHUNKS)
        ]
        zs_sl = [
            ctx.enter_context(nc.sbuf_tensor(f"zs{t}", [P, A, D], f32))
            for t in range(NT)
        ]
        stats_bn = ctx.enter_context(nc.sbuf_tensor("sbn", [P, BN_COLS], f32))
        stats_f32 = ctx.enter_context(nc.sbuf_tensor("sf", [P, F32_COLS], f32))
        # DVE dot sink: one dead column per dot (broadcast out costs the same
        # as a packed out on this DVE; distinct cols appease the race detector)
        dummy_v = ctx.enter_context(nc.sbuf_tensor("dv", [P, 64], f32))
        # ACT accum + sink in PSUM; staged to stats_f32 per chunk (no DMA
        # route out of PSUM)
        pacc = ctx.enter_context(nc.psum_tensor("pacc", [P, 64], f32))
        pdum = ctx.enter_context(nc.psum_tensor("pdum", [P, 64], f32))

        # warm-up scratch: lets each engine run one op at boot (hides the ACT
        # table load + engine start before the first data arrives)
        wtile = ctx.enter_context(nc.sbuf_tensor("wt", [P, 16], f32))
        wbn = ctx.enter_context(nc.sbuf_tensor("wbn", [P, 6], f32))
        wacc = ctx.enter_context(nc.psum_tensor("wacc", [P, 1], f32))
        wdum = ctx.enter_context(nc.psum_tensor("wdum", [P, 1], f32))

        zc_sems = [ctx.enter_context(nc.semaphore(f"dc{c}")) for c in range(CHUNKS)]
        zc0_sems = [ctx.enter_context(nc.semaphore(f"dc0r{i}")) for i in range(2)]
        zs_sems = [ctx.enter_context(nc.semaphore(f"dt{t}")) for t in range(NT)]
        t1_sems = [ctx.enter_context(nc.semaphore(f"dt1r{i}")) for i in range(2)]
        t0_sems = [ctx.enter_context(nc.semaphore(f"dt0r{i}")) for i in range(2)]
        t14_sems = [ctx.enter_context(nc.semaphore(f"dt14h{i}")) for i in range(2)]
        t15_sems = [ctx.enter_context(nc.semaphore(f"dt15h{i}")) for i in range(2)]
        sem_wu = ctx.enter_context(nc.semaphore("wu"))
        sem_vz = ctx.enter_context(nc.semaphore("vz"))   # DVE op completions
        sem_sz = ctx.enter_context(nc.semaphore("sz"))   # ACT square completions
        sem_cp = ctx.enter_context(nc.semaphore("cp"))   # ACT psum->sbuf copies
        sem_ov = ctx.enter_context(nc.semaphore("ov"))   # sync-ring output DMAs
        sem_os = ctx.enter_context(nc.semaphore("os"))   # scalar-ring output DMA

        # ---- sync: HWDGE f32 input stream, free-running ----
        # Head interleaves zc0 + t1 sub-DMAs (DVE needs zc0, ACT needs a
        # v>=1 tile ASAP); t0 follows for DVE's chunk-0 dots.
        def load(dst, src, sem):
            nc.sync.dma_start(out=dst, in_=src).then_inc(sem, 16)

        load(zc_sl[0].ap()[:, 0:1, :], zc_v[0, :, 0:1, :], zc0_sems[0])
        load(zs_sl[1].ap()[:, 0:1, :], zs_v[1, 0, :, 0:1, :], t1_sems[0])
        load(zc_sl[0].ap()[:, 1:2, :], zc_v[0, :, 1:2, :], zc0_sems[1])
        load(zs_sl[1].ap()[:, 1:2, :], zs_v[1, 0, :, 1:2, :], t1_sems[1])
        load(zc_sl[0].ap()[:, 2:4, :], zc_v[0, :, 2:4, :], zc0_sems[2])
        load(zs_sl[1].ap()[:, 2:4, :], zs_v[1, 0, :, 2:4, :], t1_sems[2])
        load(zs_sl[0].ap()[:, 0:2, :], zs_v[0, 0, :, 0:2, :], t0_sems[0])
        load(zs_sl[0].ap()[:, 2:4, :], zs_v[0, 0, :, 2:4, :], t0_sems[1])
        load(zs_sl[2].ap(), zs_v[2, 0], zs_sems[2])
        load(zs_sl[3].ap(), zs_v[3, 0], zs_sems[3])
        for c in range(1, CHUNKS):
            load(zc_sl[c].ap(), zc_v[c], zc_sems[c])
            for v in range(V):
                load(zs_sl[4 * c + v].ap(), zs_v[v, c], zs_sems[4 * c + v])

        # ---- DVE: 16 cn2 bn_stats + 64 dots ----
        def bn_cn2(c, a):
            k = 6 * (4 * c + a)
            nc.vector.bn_stats(
                out=stats_bn.ap()[:, k : k + 6],
                in_=zc_sl[c].ap()[:, a, :],
            ).then_inc(sem_vz, 1)

        def dot(c, v, a):
            k = 16 * c + 4 * v + a
            nc.vector.scalar_tensor_tensor(
                out=dummy_v.ap()[:, k : k + 1].broadcast_to((P, D)),
                in0=zc_sl[c].ap()[:, a, :],
                scalar=1.0,
                in1=zs_sl[4 * c + v].ap()[:, a, :],
                op0=mult,
                op1=mult,
                accum_out=stats_f32.ap()[:, 32 * c + 4 * v + a : 32 * c + 4 * v + a + 1],
            ).then_inc(sem_vz, 1)

        # chunk 0 (ramped head): chase sub-arrivals
        nc.vector.wait_ge(zc0_sems[0], 16)
        bn_cn2(0, 0)
        nc.vector.wait_ge(zc0_sems[1], 16)
        bn_cn2(0, 1)
        nc.vector.wait_ge(zc0_sems[2], 16)
        bn_cn2(0, 2)
        bn_cn2(0, 3)
        nc.vector.wait_ge(t1_sems[0], 16)
        dot(0, 1, 0)
        nc.vector.wait_ge(t1_sems[1], 16)
        dot(0, 1, 1)
        nc.vector.wait_ge(t1_sems[2], 16)
        dot(0, 1, 2)
        dot(0, 1, 3)
        nc.vector.wait_ge(t0_sems[0], 16)
        dot(0, 0, 0)
        dot(0, 0, 1)
        nc.vector.wait_ge(t0_sems[1], 16)
        dot(0, 0, 2)
        dot(0, 0, 3)
        for v in (2, 3):
            nc.vector.wait_ge(zs_sems[v], 16)
            for a in range(A):
                dot(0, v, a)
        for c in range(1, CHUNKS):
            nc.vector.wait_ge(zc_sems[c], 16)
            for a in range(A):
                bn_cn2(c, a)
            for v in range(V):
                nc.vector.wait_ge(zs_sems[4 * c + v], 16)
                for a in range(A):
                    dot(c, v, a)
        N_DVE = 80

        # ---- ACT: 64 sn2 squares + per-chunk psum copies ----
        def sq(c, v, a):
            k = 16 * c + 4 * v + a
            nc.scalar.activation(
                out=pdum.ap()[:, k % 64 : k % 64 + 1].broadcast_to((P, D)),
                in_=zs_sl[4 * c + v].ap()[:, a, :],
                func=mybir.ActivationFunctionType.Square,
                accum_out=pacc.ap()[:, k : k + 1],
            ).then_inc(sem_sz, 1)

        def copy_chunk(c):
            # stage this chunk's 16 PSUM accums to stats_f32; self-wait makes
            # the happens-after edge explicit for the race detector
            nc.scalar.wait_ge(sem_sz, 16 * (c + 1))
            nc.scalar.copy(
                out=stats_f32.ap()[:, 32 * c + 16 : 32 * c + 32],
                in_=pacc.ap()[:, 16 * c : 16 * c + 16],
            ).then_inc(sem_cp, 1)

        # chunk 0: t1 subs first (earliest ACT work), then t0 halves, t2, t3
        nc.scalar.wait_ge(t1_sems[0], 16)
        sq(0, 1, 0)
        nc.scalar.wait_ge(t1_sems[1], 16)
        sq(0, 1, 1)
        nc.scalar.wait_ge(t1_sems[2], 16)
        sq(0, 1, 2)
        sq(0, 1, 3)
        nc.scalar.wait_ge(t0_sems[0], 16)
        sq(0, 0, 0)
        sq(0, 0, 1)
        nc.scalar.wait_ge(t0_sems[1], 16)
        sq(0, 0, 2)
        sq(0, 0, 3)
        for v in (2, 3):
            nc.scalar.wait_ge(zs_sems[v], 16)
            for a in range(A):
                sq(0, v, a)
        copy_chunk(0)
        for c in range(1, CHUNKS):
            for v in range(V):
                nc.scalar.wait_ge(zs_sems[4 * c + v], 16)
                for a in range(A):
                    sq(c, v, a)
            copy_chunk(c)
        # late bn output on the ACT HWDGE ring (parallel to sync's). The
        # chunk-2/3 cn2 bn ops are DVE ops 41-44 and 61-64, so vz>=64
        # suffices — ACT reaches here after its own squares anyway.
        nc.scalar.wait_ge(sem_vz, 64)
        nc.scalar.dma_start(
            out=out_bn_t.ap()[:, 48:BN_COLS], in_=stats_bn.ap()[:, 48:BN_COLS]
        ).then_inc(sem_os, 16)
        nc.scalar.wait_ge(sem_os, 16)

        # ---- sync: early outputs mid-stream, late f32 at the end ----
        nc.sync.wait_ge(sem_vz, 40)   # all chunk<=1 DVE ops (20 per chunk)
        nc.sync.dma_start(
            out=out_bn_t.ap()[:, 0:48], in_=stats_bn.ap()[:, 0:48]
        ).then_inc(sem_ov, 16)
        nc.sync.wait_ge(sem_cp, 2)
        nc.sync.dma_start(
            out=out_f32_t.ap()[:, 0:64], in_=stats_f32.ap()[:, 0:64]
        ).then_inc(sem_ov, 16)
        nc.sync.wait_ge(sem_vz, N_DVE)
        nc.sync.wait_ge(sem_cp, 4)
        nc.sync.dma_start(
            out=out_f32_t.ap()[:, 64:F32_COLS], in_=stats_f32.ap()[:, 64:F32_COLS]
        ).then_inc(sem_ov, 16)
        nc.sync.wait_ge(sem_ov, 48)

    nc.compile()
    return nc


def _get_program():
    global _PROGRAM
    if _PROGRAM is None:
        _PROGRAM = _build_program()
    return _PROGRAM


def _make_in_maps(zc: np.ndarray, zs: np.ndarray):
    return [
        {
            "zc": np.ascontiguousarray(zc[i * N_LOC : (i + 1) * N_LOC]),
            "zs": np.ascontiguousarray(zs[:, i * N_LOC : (i + 1) * N_LOC]),
        }
        for i in range(N_CORES)
    ]


def _bn_sumsq(g: np.ndarray) -> np.ndarray:
    """bn_stats 6-tuple [ce, me, M2e, co, mo, M2o] -> sum of squares."""
    return g[..., 2] + g[..., 0] * g[..., 1] ** 2 + g[..., 5] + g[..., 3] * g[..., 4] ** 2


def _combine(out_bn: np.ndarray, out_f32: np.ndarray) -> tuple[float, float]:
    """out_bn: [cores, P, 96], out_f32: [cores, P, 128] -> (common, special)."""
    bn = out_bn.astype(np.float64).reshape(-1, P, BLOCKS, 6)
    f = out_f32.astype(np.float64).reshape(-1, P, CHUNKS, 2, V, A)
    cn2 = _bn_sumsq(bn).reshape(-1, P, CHUNKS, A)   # block (c,a)
    dots = f[:, :, :, 0]                            # [cores,P,c,v,a]
    sn2 = f[:, :, :, 1]

    common = cn2.sum()
    eps = 1e-12
    cn = np.maximum(np.sqrt(cn2), eps)
    sn = np.maximum(np.sqrt(sn2), eps)
    special = (dots / (cn[:, :, :, None, :] * sn)).sum()
    return common, special


def kernel(zc: np.ndarray, zs: np.ndarray) -> np.ndarray:
    from concourse.bass_utils import run_bass_kernel_spmd

    zc = np.ascontiguousarray(np.asarray(zc), dtype=np.float32)
    zs = np.ascontiguousarray(np.asarray(zs), dtype=np.float32)
    assert zc.shape == (N, D) and zs.shape == (V, N, D)

    nc = _get_program()
    res = run_bass_kernel_spmd(nc, _make_in_maps(zc, zs), core_ids=list(range(N_CORES)))
    out_bn = np.stack([r["out_bn"] for r in res.results])
    out_f32 = np.stack([r["out_f32"] for r in res.results])
    common, special = _combine(out_bn, out_f32)
    return np.asarray([-common, -special], dtype=np.float32)
